# revision 4
# baseline (speedup 1.0000x reference)
"""Trainium2 Bass kernel for nn_MidmLMHeadModelWrapper (dense transformer
attention layer: QKV proj + partial RoPE + KV-cache update + softmax
attention + output projection), tensor-parallel over heads on 8 NeuronCores.

Sharding: heads 4c..4c+3 on core c.  QKV weight columns and proj weight rows
shard with heads; the KV cache shards with heads; the final projection is a
row-sharded matmul whose partial outputs are summed on the host (the unshard
step).

Matmul dtypes: scores path (QK projections, QK^T) runs as float32r (full PE
rate, ~1e-4 relative error).  The value path (V projection, attn@V) and the
output projection run in bf16 with fp32 PSUM accumulation.  Softmax runs
without max-subtraction (scores are O(6) for any bounded mask, exp stays
finite in fp32).
"""

import numpy as np
import ml_dtypes
from contextlib import ExitStack

import concourse.bass as bass
import concourse.tile as tile
from concourse import mybir
from concourse.bass_utils import run_bass_kernel_spmd

# problem shapes (hardcoded per contract)
B, Q, D = 2, 512, 4096
H, HD = 32, 128
MAXLEN, ROT = 4096, 64
N_CORES = 8
HL = H // N_CORES          # 4 heads per core
BQ = B * Q                 # 1024
NKT = D // 128             # 32 contraction tiles over D
NST = MAXLEN // 128        # 32 seq tiles over the cache
ALPHA = 1.0 / float(HD) ** 0.5

f32 = mybir.dt.float32
f32r = mybir.dt.float32r
bf16 = mybir.dt.bfloat16
AF = mybir.ActivationFunctionType


def _split_multi_waits(nc, max_waits=1):
    """This container's walrus supports ONE inline sync-wait per instruction.
    Move excess waits onto standalone EventSemaphore instructions inserted
    immediately before, preserving per-engine program order."""
    ctr = 0
    for f in nc.m.functions:
        for bb in f.blocks:
            changed = False
            new_insts = []
            for inst in bb.instructions:
                si = inst.sync_info
                if si is not None and len(si.on_wait) > max_waits:
                    waits = list(si.on_wait)
                    extra, keep = waits[:-max_waits], waits[-max_waits:]
                    for w in extra:
                        ctr += 1
                        ev = mybir.InstEventSemaphore(
                            name=f"I-waitsplit-{ctr}", ins=[], outs=[])
                        ev.engine = inst.engine
                        ev.sync_info = mybir.SyncInfo(on_wait=[w], on_update=[])
                        new_insts.append(ev)
                    si.on_wait = keep
                    inst.sync_info = si
                    changed = True
                new_insts.append(inst)
            if changed:
                bb.instructions = new_insts
    return ctr


def build_program(step_tile: int, repeats: int = 1):
    """Emit the per-core program.  step_tile = current_step // 128."""
    nc = bass.Bass()

    hT = nc.dram_tensor("hT", [D, BQ], f32r, kind="ExternalInput")
    hTb = nc.dram_tensor("hTb", [D, BQ], bf16, kind="ExternalInput")
    wqk = nc.dram_tensor("wqk", [D, 8 * 128], f32r, kind="ExternalInput")
    wv = nc.dram_tensor("wv", [D, 512], bf16, kind="ExternalInput")
    bqk = nc.dram_tensor("bqk", [1, 8 * 128], f32r, kind="ExternalInput")
    bvb = nc.dram_tensor("bvb", [1, 512], bf16, kind="ExternalInput")
    ones = nc.dram_tensor("ones", [1, BQ], f32r, kind="ExternalInput")
    onesr = nc.dram_tensor("onesr", [1, 128], f32r, kind="ExternalInput")
    onesb = nc.dram_tensor("onesb", [128, 1], bf16, kind="ExternalInput")
    onesbr = nc.dram_tensor("onesbr", [1, 128], bf16, kind="ExternalInput")
    rmat = nc.dram_tensor("rmat", [ROT, ROT], f32, kind="ExternalInput")
    kTp = nc.dram_tensor("kTp", [B * HL, 128, MAXLEN], f32r, kind="ExternalInput")
    vp = nc.dram_tensor("vp", [B * HL, MAXLEN, HD], bf16, kind="ExternalInput")
    maskT = nc.dram_tensor("maskT", [B, MAXLEN, Q], f32, kind="ExternalInput")
    cosT = nc.dram_tensor("cosT", [B, ROT, Q], f32, kind="ExternalInput")
    sinT = nc.dram_tensor("sinT", [B, ROT, Q], f32, kind="ExternalInput")
    wp = nc.dram_tensor("wp", [HL * HD, D], bf16, kind="ExternalInput")
    outp = nc.dram_tensor("outp", [BQ, D], f32, kind="ExternalOutput")

    with tile.TileContext(nc) as tc:
        with ExitStack() as octx:
            persist = octx.enter_context(tc.tile_pool(name="persist", bufs=1))
            consts = octx.enter_context(tc.tile_pool(name="consts", bufs=1))

            onesr_t = consts.tile([1, 128], f32r, tag="onesr")
            nc.gpsimd.dma_start(onesr_t[:], onesr[:])
            onesb_t = consts.tile([128, 1], bf16, tag="onesb")
            nc.gpsimd.dma_start(onesb_t[:], onesb[:])

            for rep in range(repeats):
                _emit_once(nc, tc, persist, onesr_t, onesb_t,
                           hT, hTb, wqk, wv, bqk, bvb, ones, onesbr, rmat,
                           kTp, vp, maskT, cosT, sinT, wp, outp, step_tile)

    _split_multi_waits(nc)
    return nc


def _emit_once(nc, tc, persist, onesr_t, onesb_t,
               hT, hTb, wqk, wv, bqk, bvb, ones, onesbr, rmat,
               kTp, vp, maskT, cosT, sinT, wp, outp, step_tile):
    new_lo, new_hi = step_tile, step_tile + Q // 128

    qkT = {}    # (s, b) -> [128, Q] f32r ; s 0..3 = q heads, 4..7 = k heads
    v_new = {}  # st 0..7 -> [128, 512] bf16

    # ---------------- phase 1: projections -----------------
    with tc.tile_pool(name="ph1c", bufs=1) as cpool, \
         tc.tile_pool(name="ph1h", bufs=4) as hpool, \
         tc.tile_pool(name="ph1w", bufs=3) as wpool, \
         tc.tile_pool(name="ph1t", bufs=1) as tpool, \
         tc.tile_pool(name="ph1e", bufs=3) as epool:

        ones_t = cpool.tile([1, BQ], f32r, tag="ones")
        nc.gpsimd.dma_start(ones_t[:], ones[:])
        onesbr_t = cpool.tile([1, 128], bf16, tag="onesbr")
        nc.gpsimd.dma_start(onesbr_t[:], onesbr[:])
        rmat_t = cpool.tile([ROT, ROT], f32, tag="rmat")
        nc.gpsimd.dma_start(rmat_t[:], rmat[:])
        bqk_t = cpool.tile([1, 8 * 128], f32r, tag="bqk")
        nc.gpsimd.dma_start(bqk_t[:], bqk[:])
        bvb_t = cpool.tile([1, 512], bf16, tag="bvb")
        nc.gpsimd.dma_start(bvb_t[:], bvb[:])
        cos_t, sin_t = {}, {}
        for b in range(B):
            cos_t[b] = cpool.tile([ROT, Q], f32, tag=f"cos{b}", name=f"cos{b}")
            nc.gpsimd.dma_start(cos_t[b][:], cosT[b])
            sin_t[b] = cpool.tile([ROT, Q], f32, tag=f"sin{b}", name=f"sin{b}")
            nc.gpsimd.dma_start(sin_t[b][:], sinT[b])

        # bf16 resident hidden for the V pass: slab[p, kt*BQ + c]
        hslab = cpool.tile([128, NKT * BQ], bf16, tag="hslab")
        hbview = hTb[:].rearrange("(t p) c -> p t c", p=128)
        for j in range(8):
            nc.sync.dma_start(
                hslab[:].rearrange("p (t c) -> p t c", c=BQ)[:, j * 4:(j + 1) * 4, :],
                hbview[:, j * 4:(j + 1) * 4, :])

        # --- passes Q (s 0..3) and K (s 4..7): f32r, streamed hT ---
        for pq, srange in ((0, range(0, 4)), (1, range(4, 8))):
            t32 = {}
            with tc.tile_pool(name=f"qkps{pq}", bufs=1, space="PSUM") as qkps:
                psums = {}
                for s in srange:
                    psums[s] = qkps.tile([128, BQ], f32, tag=f"qk{s % 4}", name=f"qkps{s}")
                for kt in range(NKT):
                    ht = hpool.tile([128, BQ], f32r, tag="ht")
                    nc.sync.dma_start(ht[:], hT[kt * 128:(kt + 1) * 128, :])
                    wt = wpool.tile([128, 512], f32r, tag="wt")
                    nc.sync.dma_start(wt[:], wqk[kt * 128:(kt + 1) * 128,
                                                 pq * 512:(pq + 1) * 512])
                    for s in srange:
                        for nh in range(2):
                            nc.tensor.matmul(
                                psums[s][:, nh * 512:(nh + 1) * 512],
                                wt[:, (s % 4) * 128:(s % 4 + 1) * 128],
                                ht[:, nh * 512:(nh + 1) * 512],
                                start=(kt == 0), stop=False)
                for s in srange:
                    for nh in range(2):
                        nc.tensor.matmul(
                            psums[s][:, nh * 512:(nh + 1) * 512],
                            bqk_t[0:1, s * 128:(s + 1) * 128],
                            ones_t[0:1, nh * 512:(nh + 1) * 512],
                            start=False, stop=True)
                for s in srange:
                    for b in range(B):
                        t = tpool.tile([128, Q], f32, tag=f"t32_{s % 4}_{b}")
                        half = psums[s][:, b * 512:(b + 1) * 512]
                        if s < 4:
                            nc.scalar.mul(t[:], half, ALPHA)
                        else:
                            nc.scalar.copy(t[:], half)
                        t32[(s, b)] = t
            # RoPE (qk psums released; partner psums now fit)
            with tc.tile_pool(name=f"rope{pq}", bufs=2, space="PSUM") as rps:
                for s in srange:
                    for b in range(B):
                        t = t32[(s, b)]
                        pps = rps.tile([ROT, Q], f32, tag="pps")
                        nc.tensor.matmul(pps[:], rmat_t[:], t[0:ROT, :],
                                         start=True, stop=True)
                        ps_sin = epool.tile([ROT, Q], f32, tag="psin")
                        nc.vector.tensor_mul(ps_sin[:], sin_t[b][:], pps[:])
                        nc.vector.tensor_mul(t[0:ROT, :], t[0:ROT, :],
                                             cos_t[b][:])
                        nc.vector.tensor_add(t[0:ROT, :], t[0:ROT, :],
                                             ps_sin[:])
                        ft = persist.tile([128, Q], f32r, tag=f"qkT_{s}_{b}")
                        nc.vector.tensor_copy(ft[:], t[:])
                        qkT[(s, b)] = ft

        # --- pass V: bf16, resident hidden ---
        with tc.tile_pool(name="vps", bufs=1, space="PSUM") as vps:
            vpsums = {}
            for st in range(8):
                vpsums[st] = vps.tile([128, 512], f32, tag=f"v{st}", name=f"vps{st}")
            for kt in range(NKT):
                wt = wpool.tile([128, 512], bf16, tag="wtv")
                nc.sync.dma_start(wt[:], wv[kt * 128:(kt + 1) * 128, :])
                for st in range(8):
                    nc.tensor.matmul(
                        vpsums[st][:],
                        hslab[:, kt * BQ + st * 128: kt * BQ + (st + 1) * 128],
                        wt[:], start=(kt == 0), stop=False)
            for st in range(8):
                nc.tensor.matmul(vpsums[st][:], onesbr_t[:], bvb_t[:],
                                 start=False, stop=True)
                vt = persist.tile([128, 512], bf16, tag=f"vnew_{st}")
                nc.vector.tensor_copy(vt[:], vpsums[st][:])
                v_new[st] = vt

    # ---------------- phase 3: attention per (b, h) -----------------
    outT = {}  # bh -> [128 hd, 512 q] bf16
    with tc.tile_pool(name="katt", bufs=2) as kpool, \
         tc.tile_pool(name="vatt", bufs=2) as vpool, \
         tc.tile_pool(name="matt", bufs=1) as mpool, \
         tc.tile_pool(name="eatt", bufs=4) as epool, \
         tc.tile_pool(name="datt", bufs=2) as dpool, \
         tc.tile_pool(name="spsp", bufs=3, space="PSUM") as scps, \
         tc.tile_pool(name="apsum", bufs=2, space="PSUM") as aps, \
         tc.tile_pool(name="dpsum", bufs=1, space="PSUM") as dps_pool, \
         tc.tile_pool(name="bpsum", bufs=1, space="PSUM") as bps_pool:

        mask_slab = None
        for bh in range(B * HL):
            b, h = divmod(bh, HL)
            if h == 0:
                mask_slab = mpool.tile([128, NST * Q], f32, tag="mask")
                mview = maskT[b].rearrange("(t p) q -> p t q", p=128)
                for j in range(8):
                    nc.sync.dma_start(
                        mask_slab[:].rearrange("p (t q) -> p t q", q=Q)
                        [:, j * 4:(j + 1) * 4, :],
                        mview[:, j * 4:(j + 1) * 4, :])

            kslab = kpool.tile([128, MAXLEN], f32r, tag="kslab")
            lo, hi = new_lo * 128, new_hi * 128
            nc.sync.dma_start(kslab[:, 0:lo // 2], kTp[bh, :, 0:lo // 2])
            nc.sync.dma_start(kslab[:, lo // 2:lo], kTp[bh, :, lo // 2:lo])
            nc.sync.dma_start(kslab[:, hi:(hi + MAXLEN) // 2],
                              kTp[bh, :, hi:(hi + MAXLEN) // 2])
            nc.sync.dma_start(kslab[:, (hi + MAXLEN) // 2:],
                              kTp[bh, :, (hi + MAXLEN) // 2:])

            vslab = vpool.tile([128, NST * HD], bf16, tag="vslab")
            vview = vp[bh].rearrange("(t p) d -> p t d", p=128)
            nc.sync.dma_start(
                vslab[:].rearrange("p (t d) -> p t d", d=HD)[:, 0:new_lo, :],
                vview[:, 0:new_lo, :])
            nc.sync.dma_start(
                vslab[:].rearrange("p (t d) -> p t d", d=HD)[:, new_hi:NST, :],
                vview[:, new_hi:NST, :])

            accv = aps.tile([128, Q], f32, tag="accv")
            dps = dps_pool.tile([1, Q], f32, tag="dps")
            for kt in range(NST):
                if new_lo <= kt < new_hi:
                    k_lhsT = qkT[(4 + h, b)][:, (kt - new_lo) * 128:
                                             (kt - new_lo + 1) * 128]
                else:
                    k_lhsT = kslab[:, kt * 128:(kt + 1) * 128]
                sps = scps.tile([128, Q], f32, tag="sps")
                nc.tensor.matmul(sps[:], k_lhsT, qkT[(h, b)][:],
                                 start=True, stop=True)
                e32 = epool.tile([128, Q], f32, tag="e32")
                nc.vector.tensor_add(e32[:], mask_slab[:, kt * Q:(kt + 1) * Q],
                                     sps[:])
                ebf = epool.tile([128, Q], bf16, tag="ebf")
                nc.scalar.activation(ebf[:], e32[:], AF.Exp)
                nc.tensor.matmul(dps[:], onesb_t[:], ebf[:],
                                 start=(kt == 0), stop=(kt == NST - 1))
                if new_lo <= kt < new_hi:
                    v_lhsT = v_new[b * 4 + (kt - new_lo)][:, h * HD:(h + 1) * HD]
                else:
                    v_lhsT = vslab[:, kt * HD:(kt + 1) * HD]
                nc.tensor.matmul(accv[:], v_lhsT, ebf[:],
                                 start=(kt == 0), stop=(kt == NST - 1))

            rec = dpool.tile([1, Q], f32, tag="rec")
            nc.vector.reciprocal(rec[:], dps[:])
            rec_r = dpool.tile([1, Q], f32r, tag="recr")
            nc.vector.tensor_copy(rec_r[:], rec[:])
            bps = bps_pool.tile([128, Q], f32, tag="bps")
            nc.tensor.matmul(bps[:], onesr_t[:], rec_r[:],
                             start=True, stop=True)
            bcs = dpool.tile([128, Q], f32, tag="bcs")
            nc.scalar.copy(bcs[:], bps[:])
            ot = persist.tile([128, Q], bf16, tag=f"outT_{bh}")
            nc.vector.tensor_mul(ot[:], bcs[:], accv[:])
            outT[bh] = ot

    # ---------------- phase 4: output projection partial -----------------
    with tc.tile_pool(name="wp4", bufs=1) as wp4, \
         tc.tile_pool(name="oev4", bufs=4) as oevp, \
         tc.tile_pool(name="pps4", bufs=3, space="PSUM") as pps4:
        wpt = {}
        for kt in range(HL):
            wpt[kt] = wp4.tile([128, D], bf16, tag=f"wp{kt}", name=f"wpt{kt}")
            nc.sync.dma_start(wpt[kt][:], wp[kt * 128:(kt + 1) * 128, :])
        for b in range(B):
            for st in range(4):
                for ncn in range(8):
                    ops = pps4.tile([128, 512], f32, tag="ops")
                    for kt in range(HL):
                        nc.tensor.matmul(
                            ops[:],
                            outT[b * 4 + kt][:, st * 128:(st + 1) * 128],
                            wpt[kt][:, ncn * 512:(ncn + 1) * 512],
                            start=(kt == 0), stop=(kt == HL - 1))
                    oev = oevp.tile([128, 512], f32, tag="oev")
                    nc.scalar.copy(oev[:], ops[:])
                    nc.sync.dma_start(
                        outp[b * 512 + st * 128: b * 512 + (st + 1) * 128,
                             ncn * 512:(ncn + 1) * 512], oev[:])


# ------------------------- host side -------------------------

_PROGRAM_CACHE = {}


def _get_program(step_tile, repeats=1):
    key = (step_tile, repeats)
    if key not in _PROGRAM_CACHE:
        _PROGRAM_CACHE[key] = build_program(step_tile, repeats)
    return _PROGRAM_CACHE[key]


def prepare_inputs(hidden_states, attention_mask, freqs, position_ids,
                   past_key, past_value, w_qkv, b_qkv, w_proj, b_proj,
                   current_step, layer_idx):
    """Shard + lay out inputs for the 8 cores. Returns (in_maps, step_tile)."""
    hidden_states = np.asarray(hidden_states, dtype=np.float32)
    attention_mask = np.asarray(attention_mask, dtype=np.float32)
    freqs = np.asarray(freqs, dtype=np.float32)
    position_ids = np.asarray(position_ids)
    past_key = np.asarray(past_key, dtype=np.float32)
    past_value = np.asarray(past_value, dtype=np.float32)
    w_qkv = np.asarray(w_qkv, dtype=np.float32)
    b_qkv = np.asarray(b_qkv, dtype=np.float32)
    w_proj = np.asarray(w_proj, dtype=np.float32)
    current_step = int(current_step)
    scale = float(int(layer_idx) + 1)
    assert current_step % 128 == 0 and current_step + Q <= MAXLEN

    hTf = np.ascontiguousarray(hidden_states.reshape(BQ, D).T)        # [D, BQ]
    hTb = hTf.astype(ml_dtypes.bfloat16)
    cos = np.cos(freqs)[position_ids]                                  # [B,Q,ROT]
    sin = np.sin(freqs)[position_ids]
    cosT = np.ascontiguousarray(cos.transpose(0, 2, 1))                # [B,ROT,Q]
    sinT = np.ascontiguousarray(sin.transpose(0, 2, 1))
    maskTf = np.ascontiguousarray(
        (attention_mask[:, 0] * scale).transpose(0, 2, 1))             # [B,MAXLEN,Q]

    R = np.zeros((ROT, ROT), dtype=np.float32)
    for i in range(ROT // 2):
        R[i, i + ROT // 2] = -1.0
        R[i + ROT // 2, i] = 1.0
    rmat = np.ascontiguousarray(R.T)

    ones = np.ones((1, BQ), dtype=np.float32)
    onesr = np.ones((1, 128), dtype=np.float32)
    onesb = np.ones((128, 1), dtype=ml_dtypes.bfloat16)
    onesbr = np.ones((1, 128), dtype=ml_dtypes.bfloat16)

    in_maps = []
    for c in range(N_CORES):
        g0 = c * HL
        wqk_c = np.empty((D, 8 * 128), dtype=np.float32)
        wqk_c[:, 0:512] = w_qkv[:, g0 * HD:(g0 + HL) * HD]
        wqk_c[:, 512:1024] = w_qkv[:, D + g0 * HD: D + (g0 + HL) * HD]
        wv_c = np.ascontiguousarray(
            w_qkv[:, 2 * D + g0 * HD: 2 * D + (g0 + HL) * HD]
        ).astype(ml_dtypes.bfloat16)
        bqk_c = np.empty((1, 8 * 128), dtype=np.float32)
        bqk_c[0, 0:512] = b_qkv[g0 * HD:(g0 + HL) * HD]
        bqk_c[0, 512:1024] = b_qkv[D + g0 * HD: D + (g0 + HL) * HD]
        bvb_c = np.ascontiguousarray(
            b_qkv[2 * D + g0 * HD: 2 * D + (g0 + HL) * HD]
        ).reshape(1, 512).astype(ml_dtypes.bfloat16)
        kTp_c = np.ascontiguousarray(
            past_key[:, g0:g0 + HL].transpose(0, 1, 3, 2)
        ).reshape(B * HL, HD, MAXLEN)
        vp_c = np.ascontiguousarray(past_value[:, g0:g0 + HL]).reshape(
            B * HL, MAXLEN, HD).astype(ml_dtypes.bfloat16)
        wp_c = np.ascontiguousarray(w_proj[g0 * HD:(g0 + HL) * HD, :]).astype(
            ml_dtypes.bfloat16)
        in_maps.append(dict(
            hT=hTf, hTb=hTb, wqk=wqk_c, wv=wv_c, bqk=bqk_c, bvb=bvb_c,
            ones=ones, onesr=onesr, onesb=onesb, onesbr=onesbr, rmat=rmat,
            kTp=kTp_c, vp=vp_c, maskT=maskTf, cosT=cosT, sinT=sinT, wp=wp_c))
    return in_maps, current_step // 128


def assemble_output(results, b_proj):
    acc = np.zeros((BQ, D), dtype=np.float64)
    for r in results:
        acc += r["outp"].astype(np.float64)
    acc += np.asarray(b_proj, dtype=np.float64)[None, :]
    return acc.astype(np.float32).reshape(B, Q, D)


def kernel(**inputs):
    in_maps, step_tile = prepare_inputs(**inputs)
    nc = _get_program(step_tile)
    res = run_bass_kernel_spmd(nc, in_maps, core_ids=list(range(N_CORES)))
    return assemble_output(res.results, inputs["b_proj"])


# revision 5
# speedup vs baseline: 19.9629x; 19.9629x over previous
"""Trainium2 Bass kernel for nn_MidmLMHeadModelWrapper (dense transformer
attention layer: QKV proj + partial RoPE + KV-cache update + softmax
attention + output projection), tensor-parallel over heads on 8 NeuronCores.

Sharding: heads 4c..4c+3 on core c.  QKV weight columns and proj weight rows
shard with heads; the KV cache shards with heads; the final projection is a
row-sharded matmul whose partial outputs are summed on the host (the unshard
step).

Matmul dtypes: scores path (QK projections, QK^T) runs as float32r (full PE
rate, ~1e-4 relative error).  The value path (V projection, attn@V) and the
output projection run in bf16 with fp32 PSUM accumulation.  Softmax runs
without max-subtraction (scores are O(6) for any bounded mask, exp stays
finite in fp32).
"""

import numpy as np
import ml_dtypes
from contextlib import ExitStack

import concourse.bass as bass
import concourse.tile as tile
from concourse import mybir
from concourse.bass_utils import run_bass_kernel_spmd

# problem shapes (hardcoded per contract)
B, Q, D = 2, 512, 4096
H, HD = 32, 128
MAXLEN, ROT = 4096, 64
N_CORES = 8
HL = H // N_CORES          # 4 heads per core
BQ = B * Q                 # 1024
NKT = D // 128             # 32 contraction tiles over D
NST = MAXLEN // 128        # 32 seq tiles over the cache
ALPHA = 1.0 / float(HD) ** 0.5

f32 = mybir.dt.float32
f32r = mybir.dt.float32r
bf16 = mybir.dt.bfloat16
AF = mybir.ActivationFunctionType


def _split_multi_waits(nc, max_waits=1):
    """This container's walrus supports ONE inline sync-wait per instruction.
    Move excess waits onto standalone EventSemaphore instructions inserted
    immediately before, preserving per-engine program order."""
    ctr = 0
    for f in nc.m.functions:
        for bb in f.blocks:
            changed = False
            new_insts = []
            for inst in bb.instructions:
                si = inst.sync_info
                if si is not None and len(si.on_wait) > max_waits:
                    waits = list(si.on_wait)
                    extra, keep = waits[:-max_waits], waits[-max_waits:]
                    for w in extra:
                        ctr += 1
                        ev = mybir.InstEventSemaphore(
                            name=f"I-waitsplit-{ctr}", ins=[], outs=[])
                        ev.engine = inst.engine
                        ev.sync_info = mybir.SyncInfo(on_wait=[w], on_update=[])
                        new_insts.append(ev)
                    si.on_wait = keep
                    inst.sync_info = si
                    changed = True
                new_insts.append(inst)
            if changed:
                bb.instructions = new_insts
    return ctr


def build_program(step_tile: int, repeats: int = 1):
    """Emit the per-core program.  step_tile = current_step // 128."""
    nc = bass.Bass()

    hT = nc.dram_tensor("hT", [D, BQ], f32r, kind="ExternalInput")
    hTb = nc.dram_tensor("hTb", [D, BQ], bf16, kind="ExternalInput")
    wqk = nc.dram_tensor("wqk", [D, 8 * 128], f32r, kind="ExternalInput")
    wv = nc.dram_tensor("wv", [D, 512], bf16, kind="ExternalInput")
    bqk = nc.dram_tensor("bqk", [1, 8 * 128], f32r, kind="ExternalInput")
    bvb = nc.dram_tensor("bvb", [1, 512], bf16, kind="ExternalInput")
    ones = nc.dram_tensor("ones", [1, BQ], f32r, kind="ExternalInput")
    onesr = nc.dram_tensor("onesr", [1, 128], f32r, kind="ExternalInput")
    onesb = nc.dram_tensor("onesb", [128, 1], bf16, kind="ExternalInput")
    onesbr = nc.dram_tensor("onesbr", [1, 128], bf16, kind="ExternalInput")
    rmat = nc.dram_tensor("rmat", [ROT, ROT], f32, kind="ExternalInput")
    kTp = nc.dram_tensor("kTp", [B * HL, 128, MAXLEN], f32r, kind="ExternalInput")
    vp = nc.dram_tensor("vp", [B * HL, MAXLEN, HD], bf16, kind="ExternalInput")
    maskT = nc.dram_tensor("maskT", [B, MAXLEN, Q], f32, kind="ExternalInput")
    cosT = nc.dram_tensor("cosT", [B, ROT, Q], f32, kind="ExternalInput")
    sinT = nc.dram_tensor("sinT", [B, ROT, Q], f32, kind="ExternalInput")
    wp = nc.dram_tensor("wp", [HL * HD, D], bf16, kind="ExternalInput")
    outp = nc.dram_tensor("outp", [BQ, D], f32, kind="ExternalOutput")

    with tile.TileContext(nc) as tc:
        with ExitStack() as octx:
            persist = octx.enter_context(tc.tile_pool(name="persist", bufs=1))
            consts = octx.enter_context(tc.tile_pool(name="consts", bufs=1))

            onesr_t = consts.tile([1, 128], f32r, tag="onesr")
            nc.gpsimd.dma_start(onesr_t[:], onesr[:])
            onesb_t = consts.tile([128, 1], bf16, tag="onesb")
            nc.gpsimd.dma_start(onesb_t[:], onesb[:])

            for rep in range(repeats):
                _emit_once(nc, tc, persist, onesr_t, onesb_t,
                           hT, hTb, wqk, wv, bqk, bvb, ones, onesbr, rmat,
                           kTp, vp, maskT, cosT, sinT, wp, outp, step_tile)

    _split_multi_waits(nc)
    return nc


def _emit_once(nc, tc, persist, onesr_t, onesb_t,
               hT, hTb, wqk, wv, bqk, bvb, ones, onesbr, rmat,
               kTp, vp, maskT, cosT, sinT, wp, outp, step_tile):
    new_lo, new_hi = step_tile, step_tile + Q // 128

    qkT = {}    # (s, b) -> [128, Q] f32r ; s 0..3 = q heads, 4..7 = k heads
    v_new = {}  # st 0..7 -> [128, 512] bf16

    # ---------------- phase 1: projections -----------------
    with tc.tile_pool(name="ph1c", bufs=1) as cpool, \
         tc.tile_pool(name="ph1h", bufs=5) as hpool, \
         tc.tile_pool(name="ph1w", bufs=4) as wpool, \
         tc.tile_pool(name="ph1t", bufs=1) as tpool, \
         tc.tile_pool(name="ph1e", bufs=3) as epool:

        ones_t = cpool.tile([1, BQ], f32r, tag="ones")
        nc.gpsimd.dma_start(ones_t[:], ones[:])
        onesbr_t = cpool.tile([1, 128], bf16, tag="onesbr")
        nc.gpsimd.dma_start(onesbr_t[:], onesbr[:])
        rmat_t = cpool.tile([ROT, ROT], f32, tag="rmat")
        nc.gpsimd.dma_start(rmat_t[:], rmat[:])
        bqk_t = cpool.tile([1, 8 * 128], f32r, tag="bqk")
        nc.gpsimd.dma_start(bqk_t[:], bqk[:])
        bvb_t = cpool.tile([1, 512], bf16, tag="bvb")
        nc.gpsimd.dma_start(bvb_t[:], bvb[:])
        cos_t, sin_t = {}, {}
        for b in range(B):
            cos_t[b] = cpool.tile([ROT, Q], f32, tag=f"cos{b}", name=f"cos{b}")
            nc.gpsimd.dma_start(cos_t[b][:], cosT[b])
            sin_t[b] = cpool.tile([ROT, Q], f32, tag=f"sin{b}", name=f"sin{b}")
            nc.gpsimd.dma_start(sin_t[b][:], sinT[b])

        # bf16 resident hidden for the V pass: slab[p, kt*BQ + c]
        hslab = cpool.tile([128, NKT * BQ], bf16, tag="hslab")
        hbview = hTb[:].rearrange("(t p) c -> p t c", p=128)
        for j in range(8):
            (nc.sync if j % 2 else nc.gpsimd).dma_start(
                hslab[:].rearrange("p (t c) -> p t c", c=BQ)[:, j * 4:(j + 1) * 4, :],
                hbview[:, j * 4:(j + 1) * 4, :])

        # --- passes Q (s 0..3) and K (s 4..7): f32r, streamed hT ---
        for pq, srange in ((0, range(0, 4)), (1, range(4, 8))):
            t32 = {}
            with tc.tile_pool(name=f"qkps{pq}", bufs=1, space="PSUM") as qkps:
                psums = {}
                for s in srange:
                    psums[s] = qkps.tile([128, BQ], f32, tag=f"qk{s % 4}", name=f"qkps{s}")
                for kt in range(NKT):
                    ht = hpool.tile([128, BQ], f32r, tag="ht")
                    eng = nc.sync if kt % 2 == 0 else nc.gpsimd
                    eng2 = nc.gpsimd if kt % 2 == 0 else nc.sync
                    eng.dma_start(ht[:, 0:512], hT[kt * 128:(kt + 1) * 128, 0:512])
                    eng2.dma_start(ht[:, 512:BQ], hT[kt * 128:(kt + 1) * 128, 512:BQ])
                    wt = wpool.tile([128, 512], f32r, tag="wt")
                    eng2.dma_start(wt[:], wqk[kt * 128:(kt + 1) * 128,
                                              pq * 512:(pq + 1) * 512])
                    for s in srange:
                        for nh in range(2):
                            nc.tensor.matmul(
                                psums[s][:, nh * 512:(nh + 1) * 512],
                                wt[:, (s % 4) * 128:(s % 4 + 1) * 128],
                                ht[:, nh * 512:(nh + 1) * 512],
                                start=(kt == 0), stop=False)
                for s in srange:
                    for nh in range(2):
                        nc.tensor.matmul(
                            psums[s][:, nh * 512:(nh + 1) * 512],
                            bqk_t[0:1, s * 128:(s + 1) * 128],
                            ones_t[0:1, nh * 512:(nh + 1) * 512],
                            start=False, stop=True)
                for s in srange:
                    for b in range(B):
                        t = tpool.tile([128, Q], f32, tag=f"t32_{s % 4}_{b}")
                        half = psums[s][:, b * 512:(b + 1) * 512]
                        if s < 4:
                            nc.scalar.mul(t[:], half, ALPHA)
                        else:
                            nc.scalar.copy(t[:], half)
                        t32[(s, b)] = t
            # RoPE (qk psums released; partner psums now fit)
            with tc.tile_pool(name=f"rope{pq}", bufs=2, space="PSUM") as rps:
                for s in srange:
                    for b in range(B):
                        t = t32[(s, b)]
                        pps = rps.tile([ROT, Q], f32, tag="pps")
                        nc.tensor.matmul(pps[:], rmat_t[:], t[0:ROT, :],
                                         start=True, stop=True)
                        ps_sin = epool.tile([ROT, Q], f32, tag="psin")
                        nc.vector.tensor_mul(ps_sin[:], sin_t[b][:], pps[:])
                        nc.vector.tensor_mul(t[0:ROT, :], t[0:ROT, :],
                                             cos_t[b][:])
                        nc.vector.tensor_add(t[0:ROT, :], t[0:ROT, :],
                                             ps_sin[:])
                        ft = persist.tile([128, Q], f32r, tag=f"qkT_{s}_{b}")
                        nc.vector.tensor_copy(ft[:], t[:])
                        qkT[(s, b)] = ft

        # --- pass V: bf16, resident hidden ---
        with tc.tile_pool(name="vps", bufs=1, space="PSUM") as vps:
            vpsums = {}
            for st in range(8):
                vpsums[st] = vps.tile([128, 512], f32, tag=f"v{st}", name=f"vps{st}")
            for kt in range(NKT):
                wt = wpool.tile([128, 512], bf16, tag="wtv")
                (nc.sync if kt % 2 else nc.gpsimd).dma_start(
                    wt[:], wv[kt * 128:(kt + 1) * 128, :])
                for st in range(8):
                    nc.tensor.matmul(
                        vpsums[st][:],
                        hslab[:, kt * BQ + st * 128: kt * BQ + (st + 1) * 128],
                        wt[:], start=(kt == 0), stop=False)
            for st in range(8):
                nc.tensor.matmul(vpsums[st][:], onesbr_t[:], bvb_t[:],
                                 start=False, stop=True)
                vt = persist.tile([128, 512], bf16, tag=f"vnew_{st}")
                nc.vector.tensor_copy(vt[:], vpsums[st][:])
                v_new[st] = vt

    # ---------------- phase 3: attention per (b, h) -----------------
    outT = {}  # bh -> [128 hd, 512 q] bf16
    with tc.tile_pool(name="katt", bufs=2) as kpool, \
         tc.tile_pool(name="vatt", bufs=2) as vpool, \
         tc.tile_pool(name="matt", bufs=1) as mpool, \
         tc.tile_pool(name="eatt", bufs=4) as epool, \
         tc.tile_pool(name="datt", bufs=2) as dpool, \
         tc.tile_pool(name="spsp", bufs=3, space="PSUM") as scps, \
         tc.tile_pool(name="apsum", bufs=2, space="PSUM") as aps, \
         tc.tile_pool(name="dpsum", bufs=1, space="PSUM") as dps_pool, \
         tc.tile_pool(name="bpsum", bufs=1, space="PSUM") as bps_pool:

        mask_slab = None
        for bh in range(B * HL):
            b, h = divmod(bh, HL)
            if h == 0:
                mask_slab = mpool.tile([128, NST * Q], f32, tag="mask")
                mview = maskT[b].rearrange("(t p) q -> p t q", p=128)
                for j in range(8):
                    (nc.sync if j % 2 else nc.gpsimd).dma_start(
                        mask_slab[:].rearrange("p (t q) -> p t q", q=Q)
                        [:, j * 4:(j + 1) * 4, :],
                        mview[:, j * 4:(j + 1) * 4, :])

            kslab = kpool.tile([128, MAXLEN], f32r, tag="kslab")
            lo, hi = new_lo * 128, new_hi * 128
            for j, (a0, a1) in enumerate(
                    [(0, lo // 2), (lo // 2, lo),
                     (hi, (hi + MAXLEN) // 2), ((hi + MAXLEN) // 2, MAXLEN)]):
                m = (a0 + a1) // 2
                (nc.sync if j % 2 else nc.gpsimd).dma_start(
                    kslab[:, a0:m], kTp[bh, :, a0:m])
                (nc.gpsimd if j % 2 else nc.sync).dma_start(
                    kslab[:, m:a1], kTp[bh, :, m:a1])

            vslab = vpool.tile([128, NST * HD], bf16, tag="vslab")
            vview = vp[bh].rearrange("(t p) d -> p t d", p=128)
            vout = vslab[:].rearrange("p (t d) -> p t d", d=HD)
            for j, (a0, a1) in enumerate([(0, new_lo // 2), (new_lo // 2, new_lo),
                                          (new_hi, (new_hi + NST) // 2),
                                          ((new_hi + NST) // 2, NST)]):
                (nc.sync if j % 2 else nc.gpsimd).dma_start(
                    vout[:, a0:a1, :], vview[:, a0:a1, :])

            accv = aps.tile([128, Q], f32, tag="accv")
            dps = dps_pool.tile([1, Q], f32, tag="dps")
            for kt in range(NST):
                if new_lo <= kt < new_hi:
                    k_lhsT = qkT[(4 + h, b)][:, (kt - new_lo) * 128:
                                             (kt - new_lo + 1) * 128]
                else:
                    k_lhsT = kslab[:, kt * 128:(kt + 1) * 128]
                sps = scps.tile([128, Q], f32, tag="sps")
                nc.tensor.matmul(sps[:], k_lhsT, qkT[(h, b)][:],
                                 start=True, stop=True)
                e32 = epool.tile([128, Q], f32, tag="e32")
                nc.vector.tensor_add(e32[:], mask_slab[:, kt * Q:(kt + 1) * Q],
                                     sps[:])
                ebf = epool.tile([128, Q], bf16, tag="ebf")
                nc.scalar.activation(ebf[:], e32[:], AF.Exp)
                nc.tensor.matmul(dps[:], onesb_t[:], ebf[:],
                                 start=(kt == 0), stop=(kt == NST - 1))
                if new_lo <= kt < new_hi:
                    v_lhsT = v_new[b * 4 + (kt - new_lo)][:, h * HD:(h + 1) * HD]
                else:
                    v_lhsT = vslab[:, kt * HD:(kt + 1) * HD]
                nc.tensor.matmul(accv[:], v_lhsT, ebf[:],
                                 start=(kt == 0), stop=(kt == NST - 1))

            rec = dpool.tile([1, Q], f32, tag="rec")
            nc.vector.reciprocal(rec[:], dps[:])
            rec_r = dpool.tile([1, Q], f32r, tag="recr")
            nc.vector.tensor_copy(rec_r[:], rec[:])
            bps = bps_pool.tile([128, Q], f32, tag="bps")
            nc.tensor.matmul(bps[:], onesr_t[:], rec_r[:],
                             start=True, stop=True)
            bcs = dpool.tile([128, Q], f32, tag="bcs")
            nc.scalar.copy(bcs[:], bps[:])
            ot = persist.tile([128, Q], bf16, tag=f"outT_{bh}")
            nc.vector.tensor_mul(ot[:], bcs[:], accv[:])
            outT[bh] = ot

    # ---------------- phase 4: output projection partial -----------------
    with tc.tile_pool(name="wp4", bufs=1) as wp4, \
         tc.tile_pool(name="oev4", bufs=4) as oevp, \
         tc.tile_pool(name="pps4", bufs=3, space="PSUM") as pps4:
        wpt = {}
        for kt in range(HL):
            wpt[kt] = wp4.tile([128, D], bf16, tag=f"wp{kt}", name=f"wpt{kt}")
            (nc.sync if kt % 2 else nc.gpsimd).dma_start(
                wpt[kt][:], wp[kt * 128:(kt + 1) * 128, :])
        for b in range(B):
            for st in range(4):
                for ncn in range(8):
                    ops = pps4.tile([128, 512], f32, tag="ops")
                    for kt in range(HL):
                        nc.tensor.matmul(
                            ops[:],
                            outT[b * 4 + kt][:, st * 128:(st + 1) * 128],
                            wpt[kt][:, ncn * 512:(ncn + 1) * 512],
                            start=(kt == 0), stop=(kt == HL - 1))
                    oev = oevp.tile([128, 512], f32, tag="oev")
                    nc.scalar.copy(oev[:], ops[:])
                    (nc.sync if ncn % 2 else nc.gpsimd).dma_start(
                        outp[b * 512 + st * 128: b * 512 + (st + 1) * 128,
                             ncn * 512:(ncn + 1) * 512], oev[:])


# ------------------------- host side -------------------------

_PROGRAM_CACHE = {}


def _get_program(step_tile, repeats=1):
    key = (step_tile, repeats)
    if key not in _PROGRAM_CACHE:
        _PROGRAM_CACHE[key] = build_program(step_tile, repeats)
    return _PROGRAM_CACHE[key]


def prepare_inputs(hidden_states, attention_mask, freqs, position_ids,
                   past_key, past_value, w_qkv, b_qkv, w_proj, b_proj,
                   current_step, layer_idx):
    """Shard + lay out inputs for the 8 cores. Returns (in_maps, step_tile)."""
    hidden_states = np.asarray(hidden_states, dtype=np.float32)
    attention_mask = np.asarray(attention_mask, dtype=np.float32)
    freqs = np.asarray(freqs, dtype=np.float32)
    position_ids = np.asarray(position_ids)
    past_key = np.asarray(past_key, dtype=np.float32)
    past_value = np.asarray(past_value, dtype=np.float32)
    w_qkv = np.asarray(w_qkv, dtype=np.float32)
    b_qkv = np.asarray(b_qkv, dtype=np.float32)
    w_proj = np.asarray(w_proj, dtype=np.float32)
    current_step = int(current_step)
    scale = float(int(layer_idx) + 1)
    assert current_step % 128 == 0 and current_step + Q <= MAXLEN

    hTf = np.ascontiguousarray(hidden_states.reshape(BQ, D).T)        # [D, BQ]
    hTb = hTf.astype(ml_dtypes.bfloat16)
    cos = np.cos(freqs)[position_ids]                                  # [B,Q,ROT]
    sin = np.sin(freqs)[position_ids]
    cosT = np.ascontiguousarray(cos.transpose(0, 2, 1))                # [B,ROT,Q]
    sinT = np.ascontiguousarray(sin.transpose(0, 2, 1))
    maskTf = np.ascontiguousarray(
        (attention_mask[:, 0] * scale).transpose(0, 2, 1))             # [B,MAXLEN,Q]

    R = np.zeros((ROT, ROT), dtype=np.float32)
    for i in range(ROT // 2):
        R[i, i + ROT // 2] = -1.0
        R[i + ROT // 2, i] = 1.0
    rmat = np.ascontiguousarray(R.T)

    ones = np.ones((1, BQ), dtype=np.float32)
    onesr = np.ones((1, 128), dtype=np.float32)
    onesb = np.ones((128, 1), dtype=ml_dtypes.bfloat16)
    onesbr = np.ones((1, 128), dtype=ml_dtypes.bfloat16)

    in_maps = []
    for c in range(N_CORES):
        g0 = c * HL
        wqk_c = np.empty((D, 8 * 128), dtype=np.float32)
        wqk_c[:, 0:512] = w_qkv[:, g0 * HD:(g0 + HL) * HD]
        wqk_c[:, 512:1024] = w_qkv[:, D + g0 * HD: D + (g0 + HL) * HD]
        wv_c = np.ascontiguousarray(
            w_qkv[:, 2 * D + g0 * HD: 2 * D + (g0 + HL) * HD]
        ).astype(ml_dtypes.bfloat16)
        bqk_c = np.empty((1, 8 * 128), dtype=np.float32)
        bqk_c[0, 0:512] = b_qkv[g0 * HD:(g0 + HL) * HD]
        bqk_c[0, 512:1024] = b_qkv[D + g0 * HD: D + (g0 + HL) * HD]
        bvb_c = np.ascontiguousarray(
            b_qkv[2 * D + g0 * HD: 2 * D + (g0 + HL) * HD]
        ).reshape(1, 512).astype(ml_dtypes.bfloat16)
        kTp_c = np.ascontiguousarray(
            past_key[:, g0:g0 + HL].transpose(0, 1, 3, 2)
        ).reshape(B * HL, HD, MAXLEN)
        vp_c = np.ascontiguousarray(past_value[:, g0:g0 + HL]).reshape(
            B * HL, MAXLEN, HD).astype(ml_dtypes.bfloat16)
        wp_c = np.ascontiguousarray(w_proj[g0 * HD:(g0 + HL) * HD, :]).astype(
            ml_dtypes.bfloat16)
        in_maps.append(dict(
            hT=hTf, hTb=hTb, wqk=wqk_c, wv=wv_c, bqk=bqk_c, bvb=bvb_c,
            ones=ones, onesr=onesr, onesb=onesb, onesbr=onesbr, rmat=rmat,
            kTp=kTp_c, vp=vp_c, maskT=maskTf, cosT=cosT, sinT=sinT, wp=wp_c))
    return in_maps, current_step // 128


def assemble_output(results, b_proj):
    acc = np.zeros((BQ, D), dtype=np.float64)
    for r in results:
        acc += r["outp"].astype(np.float64)
    acc += np.asarray(b_proj, dtype=np.float64)[None, :]
    return acc.astype(np.float32).reshape(B, Q, D)


def kernel(**inputs):
    in_maps, step_tile = prepare_inputs(**inputs)
    nc = _get_program(step_tile)
    res = run_bass_kernel_spmd(nc, in_maps, core_ids=list(range(N_CORES)))
    return assemble_output(res.results, inputs["b_proj"])
